# revision 36
# baseline (speedup 1.0000x reference)
"""Trainium2 Bass kernel for nn_CDECF (graph-ODE collaborative filtering).

Contract: kernel(**inputs) takes FULL unsharded numpy inputs (as produced by
reference.setup_inputs()) and returns the FULL [8192] float32 output.

Strategy (v2 — "transposed world")
----------------------------------
Only edges with both endpoints in the batch windows contribute (the reference
scatters batch rows into node rows [0,B) and [NU,NU+B)); host preprocessing
compacts the problem to a 16384-row space (~134k edges), 2048 rows per core.

The node table lives in DRAM as [8192, 128] bf16 = row PAIRS of the logical
[16384, 64] table; each 256B dma_gather descriptor fetches one pair.  Edges
are bucketed by (out panel-pair, src-row parity) into uniform K chunks of 128
slots.  Per ODE step, per core:

  - 18x dma_gather (1024 idxs each) round-robined over 4 SWDGE queues so the
    desc-gen runs on all four Q7 cpu pairs concurrently
  - gate MLP entirely in the transposed orientation (state is kept as
    T.T [64, 2048] fp32) -- no transposes anywhere
  - scatter via swapped one-hot matmuls: LDW the gathered 64-col G half
    (stationary), move the [128 x 256] selection matrix; PSUM accumulates
    effect.T [64, 256] per panel-pair
  - Euler update + bf16 cast on DVE; one XBAR dma-transpose builds the
    row-major AllGather staging tile; AllGather republishes the table
Final scoring (sigmoid of U.I) is a trivial host-side epilogue.
"""
import sys

for _p in ("/opt/trn_rl_repo", "/root/.axon_site/_ro/trn_rl_repo"):
    if _p not in sys.path:
        sys.path.append(_p)

import numpy as np
import ml_dtypes

import concourse.bass as bass
import concourse.bacc as bacc
import concourse.mybir as mybir
import concourse.tile as tile
from concourse import bass_utils

BF16 = ml_dtypes.bfloat16

NCORES = 8
NU, NI, B, D = 50000, 20000, 8192, 64
ROWS = 2 * B            # 16384 compact rows
SLICE = ROWS // NCORES  # 2048 rows per core
PANEL = 128
NPP = 8                 # panel-pairs per core (256 rows each)
CHUNK = 128             # slots per chunk
WIDTH = 2 * PANEL       # output cols per chunk matmul
GCALL = 1024            # gather idxs per dma_gather call (ring cap per queue)
NSTEP = 3

_PROG_CACHE = {}


# ----------------------------------------------------------------------------
# Host preprocessing
# ----------------------------------------------------------------------------

def _compact_rows_user(b):
    return 2048 * (b // 1024) + (b % 1024)


def _compact_rows_item(b):
    return 2048 * (b // 1024) + 1024 + (b % 1024)


def _preprocess_edges(edge_src, edge_dst, edge_vals):
    src = np.asarray(edge_src).astype(np.int64)
    dst = np.asarray(edge_dst).astype(np.int64)
    val = np.asarray(edge_vals).astype(np.float32)

    def in_s(x):
        return (x < B) | ((x >= NU) & (x < NU + B))

    mask = in_s(src) & in_s(dst)
    s, d, v = src[mask], dst[mask], val[mask]

    def compact(ids):
        b_item = ids - NU
        return np.where(ids < B, _compact_rows_user(ids),
                        _compact_rows_item(b_item)).astype(np.int64)

    cs, cd = compact(s), compact(d)

    core = cs // SLICE
    pp = (cs % SLICE) // WIDTH          # panel-pair 0..7
    col = cs % WIDTH                    # out col within panel-pair
    q = cd % 2                          # parity of src row
    pi = cd // 2                        # gather pair index 0..8191

    cell = core * (2 * NPP) + pp * 2 + q      # 0..127
    counts = np.bincount(cell, minlength=NCORES * 2 * NPP)
    K = int(np.ceil(counts.max() / CHUNK))
    nchunk = 2 * NPP * K                       # chunks per core
    nslots = nchunk * CHUNK                    # slots per core
    assert nslots % GCALL == 0

    order = np.argsort(cell, kind="stable")
    cell_s = cell[order]
    base = np.zeros(NCORES * 2 * NPP, np.int64)
    base[1:] = np.cumsum(counts)[:-1]
    rank = np.arange(len(order)) - base[cell_s]
    core_s = cell_s // (2 * NPP)
    lcell = cell_s % (2 * NPP)                 # cell within core
    slot = lcell * K * CHUNK + rank            # slot within core

    pi_s = pi[order]
    v_s = v[order]
    col_s = col[order]

    idx_arr = np.zeros((NCORES, nslots), np.int16)
    idx_arr[core_s, slot] = pi_s.astype(np.int16)

    sel = np.zeros((NCORES, nslots, WIDTH), np.float32)
    sel[core_s, slot, col_s] = v_s
    # SBUF layout [core, 128 slot-partitions, nchunk*WIDTH cols]
    sel = sel.reshape(NCORES, nchunk, CHUNK, WIDTH).transpose(0, 2, 1, 3)
    sel = np.ascontiguousarray(sel.reshape(NCORES, CHUNK, nchunk * WIDTH))
    sel_bf = sel.astype(BF16)

    # wrapped gather indices: per call block of GCALL slots, wrapped into
    # 16 partitions: wrapped[p, s] = block_idx[s*16 + p]
    ncall = nslots // GCALL
    w = idx_arr.reshape(NCORES, ncall, GCALL // 16, 16).transpose(0, 3, 1, 2)
    gidx = np.ascontiguousarray(w.reshape(NCORES, 16, ncall * (GCALL // 16)))

    return K, nchunk, nslots, sel_bf, gidx


# ----------------------------------------------------------------------------
# Device program
# ----------------------------------------------------------------------------

def _build_program(K, nchunk, nslots, dts):
    FP32 = mybir.dt.float32
    BF = mybir.dt.bfloat16
    nc = bacc.Bacc("TRN2", target_bir_lowering=False, debug=False,
                   num_devices=NCORES, num_swdge_queues=4)

    # --- I/O -----------------------------------------------------------------
    table0 = nc.dram_tensor("table0", [ROWS // 2, 2 * D], BF,
                            kind="ExternalInput")
    slice0 = nc.dram_tensor("slice0", [D, SLICE], FP32, kind="ExternalInput")
    stage0 = nc.dram_tensor("stage0", [D, SLICE], BF, kind="ExternalInput")
    NQUART = 4
    qw = nchunk // NQUART * WIDTH
    selm_in = [nc.dram_tensor(f"selm{i}", [CHUNK, qw], BF,
                              kind="ExternalInput") for i in range(NQUART)]
    gidx_in = nc.dram_tensor("gidx", [128, nslots // 16], mybir.dt.int16,
                             kind="ExternalInput")
    w1u_in = nc.dram_tensor("w1u", [D, D], BF, kind="ExternalInput")
    w1i_in = nc.dram_tensor("w1i", [D, D], BF, kind="ExternalInput")
    w2_in = nc.dram_tensor("w2", [D, D], BF, kind="ExternalInput")
    b1_in = nc.dram_tensor("b1", [D, 1], FP32, kind="ExternalInput")
    b2_in = nc.dram_tensor("b2", [D, 1], FP32, kind="ExternalInput")
    outsl = nc.dram_tensor("outslice", [D, SLICE], FP32,
                           kind="ExternalOutput")

    # --- internal DRAM -------------------------------------------------------
    ag_in = [nc.dram_tensor(f"ag_in{s}", [SLICE, D], BF)
             for s in range(NSTEP - 1)]
    tbl_ag = [nc.dram_tensor(f"tbl_ag{s}", [ROWS // 2, 2 * D], BF,
                             addr_space="Shared") for s in range(NSTEP - 1)]
    warm_in = nc.dram_tensor("warm_in", [1, 2 * D], BF)
    warm_out = nc.dram_tensor("warm_out", [NCORES, 2 * D], BF,
                              addr_space="Shared")

    ncall = nslots // GCALL      # gather calls per step (18)
    cpc = GCALL // CHUNK         # chunks per gather call (8)
    NB = SLICE // 2              # local batch (1024)

    with tile.TileContext(nc) as tc:
        with (
            tc.tile_pool(name="cst", bufs=1) as cst,
            tc.tile_pool(name="state", bufs=1) as state,
            tc.tile_pool(name="work", bufs=2) as work,
            tc.tile_pool(name="psum", bufs=4, space="PSUM") as psum,
            tc.tile_pool(name="mpsum", bufs=2, space="PSUM") as mpsum,
        ):
            # --- persistent tiles -------------------------------------------
            selm = [cst.tile([CHUNK, qw], BF, name=f"selm{i}")
                    for i in range(NQUART)]
            gidx = cst.tile([128, nslots // 16], mybir.dt.int16)
            w1u = cst.tile([D, D], BF)
            w1i = cst.tile([D, D], BF)
            w2 = cst.tile([D, D], BF)
            b1 = cst.tile([D, 1], FP32)
            b2 = cst.tile([D, 1], FP32)
            T = [state.tile([D, SLICE], FP32, name=f"T{i}") for i in range(2)]
            G = [state.tile([CHUNK, cpc * 2 * D], BF, name=f"G{g}")
                 for g in range(ncall)]
            hT = state.tile([D, NB], BF)
            wT = state.tile([D, NB], FP32)
            dtw = state.tile([D, NB], FP32)
            stage = state.tile([D, SLICE], BF)
            agstage = state.tile([PANEL, (SLICE // PANEL) * D], BF)

            # warm the collective path and the Q7 dma_gather library while
            # the input loads stream in
            dummy_idx = cst.tile([128, 8], mybir.dt.int16, name="dummy_idx")
            dummy_g = cst.tile([CHUNK, 2 * D], BF, name="dummy_g")
            nc.vector.memset(dummy_idx[:], 0)
            nc.gpsimd.collective_compute(
                "AllGather",
                mybir.AluOpType.bypass,
                replica_groups=[list(range(NCORES))],
                ins=[warm_in.ap().opt()],
                outs=[warm_out.ap().opt()],
            )
            nc.gpsimd.dma_gather(
                out_ap=dummy_g[:].rearrange("p (c e) -> p c e", e=2 * D),
                in_ap=table0.ap(),
                idxs_ap=dummy_idx[:, :],
                num_idxs=CHUNK,
                num_idxs_reg=CHUNK,
                elem_size=2 * D,
                queue_num=0,
            )
            nc.sync.dma_start(gidx[:], gidx_in[:])
            nc.sync.dma_start(w1u[:], w1u_in[:])
            nc.sync.dma_start(w1i[:], w1i_in[:])
            nc.sync.dma_start(w2[:], w2_in[:])
            nc.sync.dma_start(b1[:], b1_in[:])
            nc.sync.dma_start(b2[:], b2_in[:])
            nc.sync.dma_start(T[0][:], slice0[:])
            nc.sync.dma_start(stage[:], stage0[:])
            for i in range(NQUART):
                nc.sync.dma_start(selm[i][:], selm_in[i][:])

            for step in range(NSTEP):
                dt = float(dts[step])
                Tcur = T[step % 2]
                Tnxt = T[(step + 1) % 2]
                tbl = table0 if step == 0 else tbl_ag[step - 1]

                # ---- gather row-pairs, 4 SWDGE queues ----------------------
                for g in range(ncall):
                    nc.gpsimd.dma_gather(
                        out_ap=G[g][:].rearrange("p (c e) -> p c e", e=2 * D),
                        in_ap=tbl.ap(),
                        idxs_ap=gidx[:, g * (GCALL // 16):(g + 1) * (GCALL // 16)],
                        num_idxs=GCALL,
                        num_idxs_reg=GCALL,
                        elem_size=2 * D,
                        queue_num=g % 4,
                    )

                # ---- gate MLP, fully transposed ----------------------------
                for hx in range(2):
                    sl = slice(hx * 512, (hx + 1) * 512)
                    hp = mpsum.tile([D, 512], FP32, tag="mlp")
                    nc.tensor.matmul(hp[:], w1u[:], stage[:, sl],
                                     start=True, stop=False)
                    nc.tensor.matmul(hp[:], w1i[:],
                                     stage[:, NB + hx * 512:NB + (hx + 1) * 512],
                                     start=False, stop=True)
                    nc.scalar.activation(hT[:, sl], hp[:],
                                         mybir.ActivationFunctionType.Relu,
                                         bias=b1[:])
                    zp = mpsum.tile([D, 512], FP32, tag="mlp")
                    nc.tensor.matmul(zp[:], w2[:], hT[:, sl],
                                     start=True, stop=True)
                    nc.scalar.activation(wT[:, sl], zp[:],
                                         mybir.ActivationFunctionType.Sigmoid,
                                         bias=b2[:])
                    nc.scalar.mul(dtw[:, sl], wT[:, sl], dt)

                # ---- scatter (swapped one-hot matmuls) + Euler update ------
                for pp in range(NPP):
                    ps = psum.tile([D, WIDTH], FP32, tag="ps")
                    for q in range(2):
                        for k in range(K):
                            t = (pp * 2 + q) * K + k
                            g, c = divmod(t, cpc)
                            qi, qt = divmod(t, nchunk // NQUART)
                            nc.tensor.matmul(
                                ps[:],
                                G[g][:, c * 2 * D + q * D:c * 2 * D + (q + 1) * D],
                                selm[qi][:, qt * WIDTH:(qt + 1) * WIDTH],
                                start=(q == 0 and k == 0),
                                stop=(q == 1 and k == K - 1),
                            )
                    colr = slice(pp * WIDTH, (pp + 1) * WIDTH)
                    wsl = slice((pp % 4) * WIDTH, (pp % 4 + 1) * WIDTH)
                    eff = work.tile([D, WIDTH], FP32, tag="eff")
                    nc.vector.tensor_tensor(eff[:], ps[:], Tcur[:, colr],
                                            op=mybir.AluOpType.subtract)
                    nc.vector.tensor_tensor(eff[:], eff[:], dtw[:, wsl],
                                            op=mybir.AluOpType.mult)
                    nc.vector.tensor_tensor(Tnxt[:, colr], Tcur[:, colr],
                                            eff[:], op=mybir.AluOpType.add)
                    if step < NSTEP - 1:
                        nc.scalar.copy(stage[:, colr], Tnxt[:, colr])

                # ---- publish updated table / final output ------------------
                if step < NSTEP - 1:
                    nc.sync.dma_start_transpose(
                        agstage[:].rearrange("p (j f) -> p j f", f=D),
                        stage[:])
                    nc.sync.dma_start(
                        ag_in[step].ap().rearrange("(j p) f -> p j f", p=PANEL),
                        agstage[:].rearrange("p (j f) -> p j f", f=D))
                    nc.gpsimd.collective_compute(
                        "AllGather",
                        mybir.AluOpType.bypass,
                        replica_groups=[list(range(NCORES))],
                        ins=[ag_in[step].ap().opt()],
                        outs=[tbl_ag[step].ap().opt()],
                    )
                else:
                    nc.sync.dma_start(outsl.ap(), Tnxt[:])

    nc.compile()
    return nc


# ----------------------------------------------------------------------------
# Entry point
# ----------------------------------------------------------------------------

def kernel(users, items, user_emb, item_emb, w1, b1, w2, b2,
           edge_src, edge_dst, edge_vals, time_steps):
    users = np.asarray(users)
    items = np.asarray(items)
    user_emb = np.asarray(user_emb, dtype=np.float32)
    item_emb = np.asarray(item_emb, dtype=np.float32)
    w1 = np.asarray(w1, dtype=np.float32)
    b1 = np.asarray(b1, dtype=np.float32)
    w2 = np.asarray(w2, dtype=np.float32)
    b2 = np.asarray(b2, dtype=np.float32)
    time_steps = np.asarray(time_steps, dtype=np.float32)
    dts = np.diff(time_steps)

    # initial compact table
    E_u = user_emb[users]
    E_i = item_emb[items]
    bidx = np.arange(B)
    rows_u = _compact_rows_user(bidx)
    rows_i = _compact_rows_item(bidx)
    table0 = np.zeros((ROWS, D), np.float32)
    table0[rows_u] = E_u
    table0[rows_i] = E_i
    table0_bf = np.ascontiguousarray(
        table0.astype(BF16).reshape(ROWS // 2, 2 * D))

    K, nchunk, nslots, sel_bf, gidx = _preprocess_edges(
        edge_src, edge_dst, edge_vals)

    key = (K, nchunk, nslots, tuple(np.round(dts, 9).tolist()))
    if key not in _PROG_CACHE:
        _PROG_CACHE[key] = _build_program(K, nchunk, nslots, dts)
    nc = _PROG_CACHE[key]

    w1u = np.ascontiguousarray(w1[:D]).astype(BF16)
    w1i = np.ascontiguousarray(w1[D:]).astype(BF16)
    w2b = w2.astype(BF16)
    b1c = np.ascontiguousarray(b1.reshape(D, 1))
    b2c = np.ascontiguousarray(b2.reshape(D, 1))

    in_maps = []
    nq = nchunk // 4 * 2 * PANEL
    for c in range(NCORES):
        sl = np.ascontiguousarray(table0[c * SLICE:(c + 1) * SLICE].T)
        m = {
            "table0": table0_bf,
            "slice0": sl,
            "stage0": sl.astype(BF16),
            "gidx": np.tile(gidx[c], (8, 1)),
            "w1u": w1u, "w1i": w1i, "w2": w2b, "b1": b1c, "b2": b2c,
        }
        for i in range(4):
            m[f"selm{i}"] = np.ascontiguousarray(
                sel_bf[c][:, i * nq:(i + 1) * nq])
        in_maps.append(m)

    res = bass_utils.run_bass_kernel_spmd(
        nc, in_maps, core_ids=list(range(NCORES)),
        trace=False)
    kernel.last_results = res

    final = np.zeros((ROWS, D), np.float32)
    for c in range(NCORES):
        final[c * SLICE:(c + 1) * SLICE] = res.results[c]["outslice"].T

    Uf = final[rows_u]
    If = final[rows_i]
    logits = np.sum(Uf * If, axis=1)
    return (1.0 / (1.0 + np.exp(-logits))).astype(np.float32)


# revision 37
# speedup vs baseline: 1.0192x; 1.0192x over previous
"""Trainium2 Bass kernel for nn_CDECF (graph-ODE collaborative filtering).

Contract: kernel(**inputs) takes FULL unsharded numpy inputs (as produced by
reference.setup_inputs()) and returns the FULL [8192] float32 output.

Strategy (v2 — "transposed world")
----------------------------------
Only edges with both endpoints in the batch windows contribute (the reference
scatters batch rows into node rows [0,B) and [NU,NU+B)); host preprocessing
compacts the problem to a 16384-row space (~134k edges), 2048 rows per core.

The node table lives in DRAM as [8192, 128] bf16 = row PAIRS of the logical
[16384, 64] table; each 256B dma_gather descriptor fetches one pair.  Edges
are bucketed by (out panel-pair, src-row parity) into uniform K chunks of 128
slots.  Per ODE step, per core:

  - 18x dma_gather (1024 idxs each) round-robined over 4 SWDGE queues so the
    desc-gen runs on all four Q7 cpu pairs concurrently
  - gate MLP entirely in the transposed orientation (state is kept as
    T.T [64, 2048] fp32) -- no transposes anywhere
  - scatter via swapped one-hot matmuls: LDW the gathered 64-col G half
    (stationary), move the [128 x 256] selection matrix; PSUM accumulates
    effect.T [64, 256] per panel-pair
  - Euler update + bf16 cast on DVE; one XBAR dma-transpose builds the
    row-major AllGather staging tile; AllGather republishes the table
Final scoring (sigmoid of U.I) is a trivial host-side epilogue.
"""
import sys

for _p in ("/opt/trn_rl_repo", "/root/.axon_site/_ro/trn_rl_repo"):
    if _p not in sys.path:
        sys.path.append(_p)

import numpy as np
import ml_dtypes

import concourse.bass as bass
import concourse.bacc as bacc
import concourse.mybir as mybir
import concourse.tile as tile
from concourse import bass_utils

BF16 = ml_dtypes.bfloat16

NCORES = 8
NU, NI, B, D = 50000, 20000, 8192, 64
ROWS = 2 * B            # 16384 compact rows
SLICE = ROWS // NCORES  # 2048 rows per core
PANEL = 128
NPP = 8                 # panel-pairs per core (256 rows each)
CHUNK = 128             # slots per chunk
WIDTH = 2 * PANEL       # output cols per chunk matmul
GCALL = 1024            # gather idxs per dma_gather call (ring cap per queue)
NSTEP = 3

_PROG_CACHE = {}


# ----------------------------------------------------------------------------
# Host preprocessing
# ----------------------------------------------------------------------------

def _compact_rows_user(b):
    return 2048 * (b // 1024) + (b % 1024)


def _compact_rows_item(b):
    return 2048 * (b // 1024) + 1024 + (b % 1024)


def _preprocess_edges(edge_src, edge_dst, edge_vals):
    src = np.asarray(edge_src).astype(np.int64)
    dst = np.asarray(edge_dst).astype(np.int64)
    val = np.asarray(edge_vals).astype(np.float32)

    def in_s(x):
        return (x < B) | ((x >= NU) & (x < NU + B))

    mask = in_s(src) & in_s(dst)
    s, d, v = src[mask], dst[mask], val[mask]

    def compact(ids):
        b_item = ids - NU
        return np.where(ids < B, _compact_rows_user(ids),
                        _compact_rows_item(b_item)).astype(np.int64)

    cs, cd = compact(s), compact(d)

    core = cs // SLICE
    pp = (cs % SLICE) // WIDTH          # panel-pair 0..7
    col = cs % WIDTH                    # out col within panel-pair
    q = cd % 2                          # parity of src row
    pi = cd // 2                        # gather pair index 0..8191

    cell = core * (2 * NPP) + pp * 2 + q      # 0..127
    counts = np.bincount(cell, minlength=NCORES * 2 * NPP)
    K = int(np.ceil(counts.max() / CHUNK))
    nchunk = 2 * NPP * K                       # chunks per core
    nslots = nchunk * CHUNK                    # slots per core
    assert nslots % GCALL == 0

    order = np.argsort(cell, kind="stable")
    cell_s = cell[order]
    base = np.zeros(NCORES * 2 * NPP, np.int64)
    base[1:] = np.cumsum(counts)[:-1]
    rank = np.arange(len(order)) - base[cell_s]
    core_s = cell_s // (2 * NPP)
    lcell = cell_s % (2 * NPP)                 # cell within core
    slot = lcell * K * CHUNK + rank            # slot within core

    pi_s = pi[order]
    v_s = v[order]
    col_s = col[order]

    idx_arr = np.zeros((NCORES, nslots), np.int16)
    idx_arr[core_s, slot] = pi_s.astype(np.int16)

    sel = np.zeros((NCORES, nslots, WIDTH), np.float32)
    sel[core_s, slot, col_s] = v_s
    # SBUF layout [core, 128 slot-partitions, nchunk*WIDTH cols]
    sel = sel.reshape(NCORES, nchunk, CHUNK, WIDTH).transpose(0, 2, 1, 3)
    sel = np.ascontiguousarray(sel.reshape(NCORES, CHUNK, nchunk * WIDTH))
    sel_bf = sel.astype(BF16)

    # wrapped gather indices: per call block of GCALL slots, wrapped into
    # 16 partitions: wrapped[p, s] = block_idx[s*16 + p]
    ncall = nslots // GCALL
    w = idx_arr.reshape(NCORES, ncall, GCALL // 16, 16).transpose(0, 3, 1, 2)
    gidx = np.ascontiguousarray(w.reshape(NCORES, 16, ncall * (GCALL // 16)))

    return K, nchunk, nslots, sel_bf, gidx


# ----------------------------------------------------------------------------
# Device program
# ----------------------------------------------------------------------------

def _build_program(K, nchunk, nslots, dts):
    FP32 = mybir.dt.float32
    BF = mybir.dt.bfloat16
    nc = bacc.Bacc("TRN2", target_bir_lowering=False, debug=False,
                   num_devices=NCORES, num_swdge_queues=4)

    # --- I/O -----------------------------------------------------------------
    table0 = nc.dram_tensor("table0", [ROWS // 2, 2 * D], BF,
                            kind="ExternalInput")
    slice0 = nc.dram_tensor("slice0", [D, SLICE], FP32, kind="ExternalInput")
    stage0 = nc.dram_tensor("stage0", [D, SLICE], BF, kind="ExternalInput")
    NQUART = 4
    qw = nchunk // NQUART * WIDTH
    selm_in = [nc.dram_tensor(f"selm{i}", [CHUNK, qw], BF,
                              kind="ExternalInput") for i in range(NQUART)]
    gidx_in = nc.dram_tensor("gidx", [128, nslots // 16], mybir.dt.int16,
                             kind="ExternalInput")
    w1u_in = nc.dram_tensor("w1u", [D, D], BF, kind="ExternalInput")
    w1i_in = nc.dram_tensor("w1i", [D, D], BF, kind="ExternalInput")
    w2_in = nc.dram_tensor("w2", [D, D], BF, kind="ExternalInput")
    b1_in = nc.dram_tensor("b1", [D, 1], FP32, kind="ExternalInput")
    b2_in = nc.dram_tensor("b2", [D, 1], FP32, kind="ExternalInput")
    outsl = nc.dram_tensor("outslice", [D, SLICE], FP32,
                           kind="ExternalOutput")

    # --- internal DRAM -------------------------------------------------------
    ag_in = [nc.dram_tensor(f"ag_in{s}", [SLICE, D], BF)
             for s in range(NSTEP - 1)]
    tbl_ag = [nc.dram_tensor(f"tbl_ag{s}", [ROWS // 2, 2 * D], BF,
                             addr_space="Shared") for s in range(NSTEP - 1)]
    warm_in = nc.dram_tensor("warm_in", [1, 2 * D], BF)
    warm_out = nc.dram_tensor("warm_out", [NCORES, 2 * D], BF,
                              addr_space="Shared")

    ncall = nslots // GCALL      # gather calls per step (18)
    cpc = GCALL // CHUNK         # chunks per gather call (8)
    NB = SLICE // 2              # local batch (1024)

    with tile.TileContext(nc) as tc:
        with (
            tc.tile_pool(name="cst", bufs=1) as cst,
            tc.tile_pool(name="state", bufs=1) as state,
            tc.tile_pool(name="work", bufs=2) as work,
            tc.tile_pool(name="psum", bufs=4, space="PSUM") as psum,
            tc.tile_pool(name="mpsum", bufs=2, space="PSUM") as mpsum,
        ):
            # --- persistent tiles -------------------------------------------
            selm = [cst.tile([CHUNK, qw], BF, name=f"selm{i}")
                    for i in range(NQUART)]
            gidx = cst.tile([128, nslots // 16], mybir.dt.int16)
            w1u = cst.tile([D, D], BF)
            w1i = cst.tile([D, D], BF)
            w2 = cst.tile([D, D], BF)
            b1 = cst.tile([D, 1], FP32)
            b2 = cst.tile([D, 1], FP32)
            T = [state.tile([D, SLICE], FP32, name=f"T{i}") for i in range(2)]
            G = [state.tile([CHUNK, cpc * 2 * D], BF, name=f"G{g}")
                 for g in range(ncall)]
            hT = state.tile([D, NB], BF)
            wT = state.tile([D, NB], FP32)
            dtw = state.tile([D, NB], FP32)
            stage = state.tile([D, SLICE], BF)
            agstage = state.tile([PANEL, (SLICE // PANEL) * D], BF)

            # warm the collective path and the Q7 dma_gather library while
            # the input loads stream in
            dummy_idx = cst.tile([128, 8], mybir.dt.int16, name="dummy_idx")
            dummy_g = cst.tile([CHUNK, 2 * D], BF, name="dummy_g")
            nc.vector.memset(dummy_idx[:], 0)
            nc.gpsimd.collective_compute(
                "AllGather",
                mybir.AluOpType.bypass,
                replica_groups=[list(range(NCORES))],
                ins=[warm_in.ap().opt()],
                outs=[warm_out.ap().opt()],
            )
            nc.gpsimd.dma_gather(
                out_ap=dummy_g[:].rearrange("p (c e) -> p c e", e=2 * D),
                in_ap=table0.ap(),
                idxs_ap=dummy_idx[:, :],
                num_idxs=CHUNK,
                num_idxs_reg=CHUNK,
                elem_size=2 * D,
                queue_num=0,
            )
            nc.sync.dma_start(gidx[:], gidx_in[:])
            nc.sync.dma_start(w1u[:], w1u_in[:])
            nc.sync.dma_start(w1i[:], w1i_in[:])
            nc.sync.dma_start(w2[:], w2_in[:])
            nc.sync.dma_start(b1[:], b1_in[:])
            nc.sync.dma_start(b2[:], b2_in[:])
            nc.sync.dma_start(T[0][:], slice0[:])
            nc.sync.dma_start(stage[:], stage0[:])
            for i in range(NQUART):
                nc.scalar.dma_start(selm[i][:], selm_in[i][:])

            for step in range(NSTEP):
                dt = float(dts[step])
                Tcur = T[step % 2]
                Tnxt = T[(step + 1) % 2]
                tbl = table0 if step == 0 else tbl_ag[step - 1]

                # ---- gather row-pairs, 4 SWDGE queues ----------------------
                # trailing calls are split in half across the otherwise idle
                # queues so every queue generates the same descriptor count
                nfull = 4 * (ncall // 4)
                for g in range(nfull):
                    nc.gpsimd.dma_gather(
                        out_ap=G[g][:].rearrange("p (c e) -> p c e", e=2 * D),
                        in_ap=tbl.ap(),
                        idxs_ap=gidx[:, g * (GCALL // 16):(g + 1) * (GCALL // 16)],
                        num_idxs=GCALL,
                        num_idxs_reg=GCALL,
                        elem_size=2 * D,
                        queue_num=g % 4,
                    )
                for g in range(nfull, ncall):
                    r = g - nfull
                    for h in range(2):
                        nc.gpsimd.dma_gather(
                            out_ap=G[g][:].rearrange(
                                "p (c e) -> p c e",
                                e=2 * D)[:, 4 * h:4 * h + 4, :],
                            in_ap=tbl.ap(),
                            idxs_ap=gidx[:, g * (GCALL // 16) + 32 * h:
                                         g * (GCALL // 16) + 32 * h + 32],
                            num_idxs=GCALL // 2,
                            num_idxs_reg=GCALL // 2,
                            elem_size=2 * D,
                            queue_num=r + 2 * h,
                        )

                # ---- gate MLP, fully transposed ----------------------------
                for hx in range(2):
                    sl = slice(hx * 512, (hx + 1) * 512)
                    hp = mpsum.tile([D, 512], FP32, tag="mlp")
                    nc.tensor.matmul(hp[:], w1u[:], stage[:, sl],
                                     start=True, stop=False)
                    nc.tensor.matmul(hp[:], w1i[:],
                                     stage[:, NB + hx * 512:NB + (hx + 1) * 512],
                                     start=False, stop=True)
                    nc.scalar.activation(hT[:, sl], hp[:],
                                         mybir.ActivationFunctionType.Relu,
                                         bias=b1[:])
                    zp = mpsum.tile([D, 512], FP32, tag="mlp")
                    nc.tensor.matmul(zp[:], w2[:], hT[:, sl],
                                     start=True, stop=True)
                    nc.scalar.activation(wT[:, sl], zp[:],
                                         mybir.ActivationFunctionType.Sigmoid,
                                         bias=b2[:])
                    nc.scalar.mul(dtw[:, sl], wT[:, sl], dt)

                # ---- scatter (swapped one-hot matmuls) + Euler update ------
                for pp in range(NPP):
                    ps = psum.tile([D, WIDTH], FP32, tag="ps")
                    for q in range(2):
                        for k in range(K):
                            t = (pp * 2 + q) * K + k
                            g, c = divmod(t, cpc)
                            qi, qt = divmod(t, nchunk // NQUART)
                            nc.tensor.matmul(
                                ps[:],
                                G[g][:, c * 2 * D + q * D:c * 2 * D + (q + 1) * D],
                                selm[qi][:, qt * WIDTH:(qt + 1) * WIDTH],
                                start=(q == 0 and k == 0),
                                stop=(q == 1 and k == K - 1),
                            )
                    colr = slice(pp * WIDTH, (pp + 1) * WIDTH)
                    wsl = slice((pp % 4) * WIDTH, (pp % 4 + 1) * WIDTH)
                    eff = work.tile([D, WIDTH], FP32, tag="eff")
                    nc.vector.tensor_tensor(eff[:], ps[:], Tcur[:, colr],
                                            op=mybir.AluOpType.subtract)
                    nc.vector.tensor_tensor(eff[:], eff[:], dtw[:, wsl],
                                            op=mybir.AluOpType.mult)
                    nc.vector.tensor_tensor(Tnxt[:, colr], Tcur[:, colr],
                                            eff[:], op=mybir.AluOpType.add)
                    if step < NSTEP - 1:
                        nc.scalar.copy(stage[:, colr], Tnxt[:, colr])
                    else:
                        nc.sync.dma_start(outsl.ap()[:, colr], Tnxt[:, colr])

                # ---- publish updated table / final output ------------------
                if step < NSTEP - 1:
                    nc.sync.dma_start_transpose(
                        agstage[:].rearrange("p (j f) -> p j f", f=D),
                        stage[:])
                    nc.sync.dma_start(
                        ag_in[step].ap().rearrange("(j p) f -> p j f", p=PANEL),
                        agstage[:].rearrange("p (j f) -> p j f", f=D))
                    nc.gpsimd.collective_compute(
                        "AllGather",
                        mybir.AluOpType.bypass,
                        replica_groups=[list(range(NCORES))],
                        ins=[ag_in[step].ap().opt()],
                        outs=[tbl_ag[step].ap().opt()],
                    )


    nc.compile()
    return nc


# ----------------------------------------------------------------------------
# Entry point
# ----------------------------------------------------------------------------

def kernel(users, items, user_emb, item_emb, w1, b1, w2, b2,
           edge_src, edge_dst, edge_vals, time_steps):
    users = np.asarray(users)
    items = np.asarray(items)
    user_emb = np.asarray(user_emb, dtype=np.float32)
    item_emb = np.asarray(item_emb, dtype=np.float32)
    w1 = np.asarray(w1, dtype=np.float32)
    b1 = np.asarray(b1, dtype=np.float32)
    w2 = np.asarray(w2, dtype=np.float32)
    b2 = np.asarray(b2, dtype=np.float32)
    time_steps = np.asarray(time_steps, dtype=np.float32)
    dts = np.diff(time_steps)

    # initial compact table
    E_u = user_emb[users]
    E_i = item_emb[items]
    bidx = np.arange(B)
    rows_u = _compact_rows_user(bidx)
    rows_i = _compact_rows_item(bidx)
    table0 = np.zeros((ROWS, D), np.float32)
    table0[rows_u] = E_u
    table0[rows_i] = E_i
    table0_bf = np.ascontiguousarray(
        table0.astype(BF16).reshape(ROWS // 2, 2 * D))

    K, nchunk, nslots, sel_bf, gidx = _preprocess_edges(
        edge_src, edge_dst, edge_vals)

    key = (K, nchunk, nslots, tuple(np.round(dts, 9).tolist()))
    if key not in _PROG_CACHE:
        _PROG_CACHE[key] = _build_program(K, nchunk, nslots, dts)
    nc = _PROG_CACHE[key]

    w1u = np.ascontiguousarray(w1[:D]).astype(BF16)
    w1i = np.ascontiguousarray(w1[D:]).astype(BF16)
    w2b = w2.astype(BF16)
    b1c = np.ascontiguousarray(b1.reshape(D, 1))
    b2c = np.ascontiguousarray(b2.reshape(D, 1))

    in_maps = []
    nq = nchunk // 4 * 2 * PANEL
    for c in range(NCORES):
        sl = np.ascontiguousarray(table0[c * SLICE:(c + 1) * SLICE].T)
        m = {
            "table0": table0_bf,
            "slice0": sl,
            "stage0": sl.astype(BF16),
            "gidx": np.tile(gidx[c], (8, 1)),
            "w1u": w1u, "w1i": w1i, "w2": w2b, "b1": b1c, "b2": b2c,
        }
        for i in range(4):
            m[f"selm{i}"] = np.ascontiguousarray(
                sel_bf[c][:, i * nq:(i + 1) * nq])
        in_maps.append(m)

    res = bass_utils.run_bass_kernel_spmd(
        nc, in_maps, core_ids=list(range(NCORES)),
        trace=False)
    kernel.last_results = res

    final = np.zeros((ROWS, D), np.float32)
    for c in range(NCORES):
        final[c * SLICE:(c + 1) * SLICE] = res.results[c]["outslice"].T

    Uf = final[rows_u]
    If = final[rows_i]
    logits = np.sum(Uf * If, axis=1)
    return (1.0 / (1.0 + np.exp(-logits))).astype(np.float32)


# revision 38
# speedup vs baseline: 1.0460x; 1.0263x over previous
"""Trainium2 Bass kernel for nn_CDECF (graph-ODE collaborative filtering).

Contract: kernel(**inputs) takes FULL unsharded numpy inputs (as produced by
reference.setup_inputs()) and returns the FULL [8192] float32 output.

Strategy (v2 — "transposed world")
----------------------------------
Only edges with both endpoints in the batch windows contribute (the reference
scatters batch rows into node rows [0,B) and [NU,NU+B)); host preprocessing
compacts the problem to a 16384-row space (~134k edges), 2048 rows per core.

The node table lives in DRAM as [8192, 128] bf16 = row PAIRS of the logical
[16384, 64] table; each 256B dma_gather descriptor fetches one pair.  Edges
are bucketed by (out panel-pair, src-row parity) into uniform K chunks of 128
slots.  Per ODE step, per core:

  - 18x dma_gather (1024 idxs each) round-robined over 4 SWDGE queues so the
    desc-gen runs on all four Q7 cpu pairs concurrently
  - gate MLP entirely in the transposed orientation (state is kept as
    T.T [64, 2048] fp32) -- no transposes anywhere
  - scatter via swapped one-hot matmuls: LDW the gathered 64-col G half
    (stationary), move the [128 x 256] selection matrix; PSUM accumulates
    effect.T [64, 256] per panel-pair
  - Euler update + bf16 cast on DVE; one XBAR dma-transpose builds the
    row-major AllGather staging tile; AllGather republishes the table
Final scoring (sigmoid of U.I) is a trivial host-side epilogue.
"""
import sys

for _p in ("/opt/trn_rl_repo", "/root/.axon_site/_ro/trn_rl_repo"):
    if _p not in sys.path:
        sys.path.append(_p)

import numpy as np
import ml_dtypes

import concourse.bass as bass
import concourse.bacc as bacc
import concourse.mybir as mybir
import concourse.tile as tile
from concourse import bass_utils

BF16 = ml_dtypes.bfloat16

NCORES = 8
NU, NI, B, D = 50000, 20000, 8192, 64
ROWS = 2 * B            # 16384 compact rows
SLICE = ROWS // NCORES  # 2048 rows per core
PANEL = 128
NPP = 8                 # panel-pairs per core (256 rows each)
CHUNK = 128             # slots per chunk
WIDTH = 2 * PANEL       # output cols per chunk matmul
GCALL = 1024            # gather idxs per dma_gather call (ring cap per queue)
NSTEP = 3

_PROG_CACHE = {}


# ----------------------------------------------------------------------------
# Host preprocessing
# ----------------------------------------------------------------------------

def _compact_rows_user(b):
    return 2048 * (b // 1024) + (b % 1024)


def _compact_rows_item(b):
    return 2048 * (b // 1024) + 1024 + (b % 1024)


def _preprocess_edges(edge_src, edge_dst, edge_vals):
    src = np.asarray(edge_src).astype(np.int64)
    dst = np.asarray(edge_dst).astype(np.int64)
    val = np.asarray(edge_vals).astype(np.float32)

    def in_s(x):
        return (x < B) | ((x >= NU) & (x < NU + B))

    mask = in_s(src) & in_s(dst)
    s, d, v = src[mask], dst[mask], val[mask]

    def compact(ids):
        b_item = ids - NU
        return np.where(ids < B, _compact_rows_user(ids),
                        _compact_rows_item(b_item)).astype(np.int64)

    cs, cd = compact(s), compact(d)

    core = cs // SLICE
    pp = (cs % SLICE) // WIDTH          # panel-pair 0..7
    col = cs % WIDTH                    # out col within panel-pair
    q = cd % 2                          # parity of src row
    pi = cd // 2                        # gather pair index 0..8191

    cell = core * (2 * NPP) + pp * 2 + q      # 0..127
    counts = np.bincount(cell, minlength=NCORES * 2 * NPP)
    K = int(np.ceil(counts.max() / CHUNK))
    nchunk = 2 * NPP * K                       # chunks per core
    nslots = nchunk * CHUNK                    # slots per core
    assert nslots % GCALL == 0

    order = np.argsort(cell, kind="stable")
    cell_s = cell[order]
    base = np.zeros(NCORES * 2 * NPP, np.int64)
    base[1:] = np.cumsum(counts)[:-1]
    rank = np.arange(len(order)) - base[cell_s]
    core_s = cell_s // (2 * NPP)
    lcell = cell_s % (2 * NPP)                 # cell within core
    slot = lcell * K * CHUNK + rank            # slot within core

    pi_s = pi[order]
    v_s = v[order]
    col_s = col[order]

    idx_arr = np.zeros((NCORES, nslots), np.int16)
    idx_arr[core_s, slot] = pi_s.astype(np.int16)

    sel = np.zeros((NCORES, nslots, WIDTH), np.float32)
    sel[core_s, slot, col_s] = v_s
    # SBUF layout [core, 128 slot-partitions, nchunk*WIDTH cols]
    sel = sel.reshape(NCORES, nchunk, CHUNK, WIDTH).transpose(0, 2, 1, 3)
    sel = np.ascontiguousarray(sel.reshape(NCORES, CHUNK, nchunk * WIDTH))
    sel_bf = sel.astype(BF16)

    # wrapped gather indices: per call block of GCALL slots, wrapped into
    # 16 partitions: wrapped[p, s] = block_idx[s*16 + p]
    ncall = nslots // GCALL
    w = idx_arr.reshape(NCORES, ncall, GCALL // 16, 16).transpose(0, 3, 1, 2)
    gidx = np.ascontiguousarray(w.reshape(NCORES, 16, ncall * (GCALL // 16)))

    return K, nchunk, nslots, sel_bf, gidx


# ----------------------------------------------------------------------------
# Device program
# ----------------------------------------------------------------------------

def _build_program(K, nchunk, nslots, dts):
    FP32 = mybir.dt.float32
    BF = mybir.dt.bfloat16
    nc = bacc.Bacc("TRN2", target_bir_lowering=False, debug=False,
                   num_devices=NCORES, num_swdge_queues=4)

    # --- I/O -----------------------------------------------------------------
    table0 = nc.dram_tensor("table0", [ROWS // 2, 2 * D], BF,
                            kind="ExternalInput")
    slice0 = nc.dram_tensor("slice0", [D, SLICE], FP32, kind="ExternalInput")
    stage0 = nc.dram_tensor("stage0", [D, SLICE], BF, kind="ExternalInput")
    NQUART = 4
    qw = nchunk // NQUART * WIDTH
    selm_in = [nc.dram_tensor(f"selm{i}", [CHUNK, qw], BF,
                              kind="ExternalInput") for i in range(NQUART)]
    gidx_in = nc.dram_tensor("gidx", [128, nslots // 16], mybir.dt.int16,
                             kind="ExternalInput")
    w1u_in = nc.dram_tensor("w1u", [D, D], BF, kind="ExternalInput")
    w1i_in = nc.dram_tensor("w1i", [D, D], BF, kind="ExternalInput")
    w2_in = nc.dram_tensor("w2", [D, D], BF, kind="ExternalInput")
    b1_in = nc.dram_tensor("b1", [D, 1], FP32, kind="ExternalInput")
    b2_in = nc.dram_tensor("b2", [D, 1], FP32, kind="ExternalInput")
    outsl = nc.dram_tensor("outslice", [D, SLICE], FP32,
                           kind="ExternalOutput")

    # --- internal DRAM -------------------------------------------------------
    ag_in = [nc.dram_tensor(f"ag_in{s}", [SLICE, D], BF)
             for s in range(NSTEP - 1)]
    tbl_ag = [nc.dram_tensor(f"tbl_ag{s}", [ROWS // 2, 2 * D], BF,
                             addr_space="Shared") for s in range(NSTEP - 1)]
    warm_in = nc.dram_tensor("warm_in", [1, 2 * D], BF)
    warm_out = nc.dram_tensor("warm_out", [NCORES, 2 * D], BF,
                              addr_space="Shared")

    ncall = nslots // GCALL      # gather calls per step (18)
    cpc = GCALL // CHUNK         # chunks per gather call (8)
    NB = SLICE // 2              # local batch (1024)

    with tile.TileContext(nc) as tc:
        with (
            tc.tile_pool(name="cst", bufs=1) as cst,
            tc.tile_pool(name="state", bufs=1) as state,
            tc.tile_pool(name="work", bufs=2) as work,
            tc.tile_pool(name="psum", bufs=4, space="PSUM") as psum,
            tc.tile_pool(name="mpsum", bufs=2, space="PSUM") as mpsum,
        ):
            # --- persistent tiles -------------------------------------------
            selm = [cst.tile([CHUNK, qw], BF, name=f"selm{i}")
                    for i in range(NQUART)]
            gidx = cst.tile([128, nslots // 16], mybir.dt.int16)
            w1u = cst.tile([D, D], BF)
            w1i = cst.tile([D, D], BF)
            w2 = cst.tile([D, D], BF)
            b1 = cst.tile([D, 1], FP32)
            b2 = cst.tile([D, 1], FP32)
            T = [state.tile([D, SLICE], FP32, name=f"T{i}") for i in range(2)]
            G = [state.tile([CHUNK, cpc * 2 * D], BF, name=f"G{g}")
                 for g in range(ncall)]
            hT = state.tile([D, NB], BF)
            wT = state.tile([D, NB], FP32)
            dtw = state.tile([D, NB], FP32)
            stage = state.tile([D, SLICE], BF)
            agstage = state.tile([PANEL, (SLICE // PANEL) * D], BF)

            # warm the collective path and the Q7 dma_gather library while
            # the input loads stream in
            dummy_idx = cst.tile([128, 8], mybir.dt.int16, name="dummy_idx")
            dummy_g = cst.tile([CHUNK, 2 * D], BF, name="dummy_g")
            nc.vector.memset(dummy_idx[:], 0)
            nc.gpsimd.collective_compute(
                "AllGather",
                mybir.AluOpType.bypass,
                replica_groups=[list(range(NCORES))],
                ins=[warm_in.ap().opt()],
                outs=[warm_out.ap().opt()],
            )
            nc.gpsimd.dma_gather(
                out_ap=dummy_g[:].rearrange("p (c e) -> p c e", e=2 * D),
                in_ap=table0.ap(),
                idxs_ap=dummy_idx[:, :],
                num_idxs=CHUNK,
                num_idxs_reg=CHUNK,
                elem_size=2 * D,
                queue_num=0,
            )
            nc.sync.dma_start(gidx[:], gidx_in[:])
            nc.sync.dma_start(w1u[:], w1u_in[:])
            nc.sync.dma_start(w1i[:], w1i_in[:])
            nc.sync.dma_start(w2[:], w2_in[:])
            nc.sync.dma_start(b1[:], b1_in[:])
            nc.sync.dma_start(b2[:], b2_in[:])
            nc.sync.dma_start(T[0][:], slice0[:])
            nc.sync.dma_start(stage[:], stage0[:])
            for i in range(NQUART):
                nc.sync.dma_start(selm[i][:], selm_in[i][:])

            for step in range(NSTEP):
                dt = float(dts[step])
                Tcur = T[step % 2]
                Tnxt = T[(step + 1) % 2]
                tbl = table0 if step == 0 else tbl_ag[step - 1]

                # ---- gather row-pairs, 4 SWDGE queues ----------------------
                for g in range(ncall):
                    nc.gpsimd.dma_gather(
                        out_ap=G[g][:].rearrange("p (c e) -> p c e", e=2 * D),
                        in_ap=tbl.ap(),
                        idxs_ap=gidx[:, g * (GCALL // 16):(g + 1) * (GCALL // 16)],
                        num_idxs=GCALL,
                        num_idxs_reg=GCALL,
                        elem_size=2 * D,
                        queue_num=g % 4,
                    )

                # ---- gate MLP, fully transposed ----------------------------
                for hx in range(2):
                    sl = slice(hx * 512, (hx + 1) * 512)
                    hp = mpsum.tile([D, 512], FP32, tag="mlp")
                    nc.tensor.matmul(hp[:], w1u[:], stage[:, sl],
                                     start=True, stop=False)
                    nc.tensor.matmul(hp[:], w1i[:],
                                     stage[:, NB + hx * 512:NB + (hx + 1) * 512],
                                     start=False, stop=True)
                    nc.scalar.activation(hT[:, sl], hp[:],
                                         mybir.ActivationFunctionType.Relu,
                                         bias=b1[:])
                    zp = mpsum.tile([D, 512], FP32, tag="mlp")
                    nc.tensor.matmul(zp[:], w2[:], hT[:, sl],
                                     start=True, stop=True)
                    nc.scalar.activation(wT[:, sl], zp[:],
                                         mybir.ActivationFunctionType.Sigmoid,
                                         bias=b2[:])
                    nc.scalar.mul(dtw[:, sl], wT[:, sl], dt)

                # ---- scatter (swapped one-hot matmuls) + Euler update ------
                for pp in range(NPP):
                    ps = psum.tile([D, WIDTH], FP32, tag="ps")
                    for q in range(2):
                        for k in range(K):
                            t = (pp * 2 + q) * K + k
                            g, c = divmod(t, cpc)
                            qi, qt = divmod(t, nchunk // NQUART)
                            nc.tensor.matmul(
                                ps[:],
                                G[g][:, c * 2 * D + q * D:c * 2 * D + (q + 1) * D],
                                selm[qi][:, qt * WIDTH:(qt + 1) * WIDTH],
                                start=(q == 0 and k == 0),
                                stop=(q == 1 and k == K - 1),
                            )
                    colr = slice(pp * WIDTH, (pp + 1) * WIDTH)
                    wsl = slice((pp % 4) * WIDTH, (pp % 4 + 1) * WIDTH)
                    eff = work.tile([D, WIDTH], FP32, tag="eff")
                    nc.vector.tensor_tensor(eff[:], ps[:], Tcur[:, colr],
                                            op=mybir.AluOpType.subtract)
                    nc.vector.tensor_tensor(eff[:], eff[:], dtw[:, wsl],
                                            op=mybir.AluOpType.mult)
                    nc.vector.tensor_tensor(Tnxt[:, colr], Tcur[:, colr],
                                            eff[:], op=mybir.AluOpType.add)
                    if step < NSTEP - 1:
                        nc.scalar.copy(stage[:, colr], Tnxt[:, colr])

                # ---- publish updated table / final output ------------------
                if step < NSTEP - 1:
                    nc.sync.dma_start_transpose(
                        agstage[:].rearrange("p (j f) -> p j f", f=D),
                        stage[:])
                    nc.sync.dma_start(
                        ag_in[step].ap().rearrange("(j p) f -> p j f", p=PANEL),
                        agstage[:].rearrange("p (j f) -> p j f", f=D))
                    nc.gpsimd.collective_compute(
                        "AllGather",
                        mybir.AluOpType.bypass,
                        replica_groups=[list(range(NCORES))],
                        ins=[ag_in[step].ap().opt()],
                        outs=[tbl_ag[step].ap().opt()],
                    )
                else:
                    nc.sync.dma_start(outsl.ap(), Tnxt[:])

    nc.compile()
    return nc


# ----------------------------------------------------------------------------
# Entry point
# ----------------------------------------------------------------------------

def kernel(users, items, user_emb, item_emb, w1, b1, w2, b2,
           edge_src, edge_dst, edge_vals, time_steps):
    users = np.asarray(users)
    items = np.asarray(items)
    user_emb = np.asarray(user_emb, dtype=np.float32)
    item_emb = np.asarray(item_emb, dtype=np.float32)
    w1 = np.asarray(w1, dtype=np.float32)
    b1 = np.asarray(b1, dtype=np.float32)
    w2 = np.asarray(w2, dtype=np.float32)
    b2 = np.asarray(b2, dtype=np.float32)
    time_steps = np.asarray(time_steps, dtype=np.float32)
    dts = np.diff(time_steps)

    # initial compact table
    E_u = user_emb[users]
    E_i = item_emb[items]
    bidx = np.arange(B)
    rows_u = _compact_rows_user(bidx)
    rows_i = _compact_rows_item(bidx)
    table0 = np.zeros((ROWS, D), np.float32)
    table0[rows_u] = E_u
    table0[rows_i] = E_i
    table0_bf = np.ascontiguousarray(
        table0.astype(BF16).reshape(ROWS // 2, 2 * D))

    K, nchunk, nslots, sel_bf, gidx = _preprocess_edges(
        edge_src, edge_dst, edge_vals)

    key = (K, nchunk, nslots, tuple(np.round(dts, 9).tolist()))
    if key not in _PROG_CACHE:
        _PROG_CACHE[key] = _build_program(K, nchunk, nslots, dts)
    nc = _PROG_CACHE[key]

    w1u = np.ascontiguousarray(w1[:D]).astype(BF16)
    w1i = np.ascontiguousarray(w1[D:]).astype(BF16)
    w2b = w2.astype(BF16)
    b1c = np.ascontiguousarray(b1.reshape(D, 1))
    b2c = np.ascontiguousarray(b2.reshape(D, 1))

    in_maps = []
    nq = nchunk // 4 * 2 * PANEL
    for c in range(NCORES):
        sl = np.ascontiguousarray(table0[c * SLICE:(c + 1) * SLICE].T)
        m = {
            "table0": table0_bf,
            "slice0": sl,
            "stage0": sl.astype(BF16),
            "gidx": np.tile(gidx[c], (8, 1)),
            "w1u": w1u, "w1i": w1i, "w2": w2b, "b1": b1c, "b2": b2c,
        }
        for i in range(4):
            m[f"selm{i}"] = np.ascontiguousarray(
                sel_bf[c][:, i * nq:(i + 1) * nq])
        in_maps.append(m)

    res = bass_utils.run_bass_kernel_spmd(
        nc, in_maps, core_ids=list(range(NCORES)),
        trace=False)
    kernel.last_results = res

    final = np.zeros((ROWS, D), np.float32)
    for c in range(NCORES):
        final[c * SLICE:(c + 1) * SLICE] = res.results[c]["outslice"].T

    Uf = final[rows_u]
    If = final[rows_i]
    logits = np.sum(Uf * If, axis=1)
    return (1.0 / (1.0 + np.exp(-logits))).astype(np.float32)
